# revision 5
# baseline (speedup 1.0000x reference)
"""Trainium2 Bass kernel for the BERT-NED span-scoring module.

Reference computation (shapes hardcoded from the problem spec):
    sent_embedding      (64, 512, 768)  f32
    entity_embedding    (64, 24, 30, 1024) f32
    entity_embedding_mask (64, 24, 30)  bool
    start/end_span_idx  (64, 24)        int
    W_span              (1536, 1024)    f32
    b_span              (1024,)         f32

    start_w/end_w = gather sentence rows at span indices (-1 -> zero row)
    pair  = concat([start_w, end_w], -1)            (B, M, 1536)
    alias = pair @ W_span + b_span                  (B, M, 1024)
    ent   = where(mask, 0, entity_embedding)        (B, M, 30, 1024)
    final_scores = einsum('bmh,bmkh->bmk', alias, ent)
    returns (final_scores, ent)

Strategy: pure data parallel over batch B (8 batches per core, 8 cores).
The index gather is tiny (64*24 rows) and data-dependent, so it runs on
host; the device does the heavy work: the span projection matmul on PE
(bias folded in as an extra contraction row, K padded 1536->1664 so every
matmul keeps K=128 — rank-1 f32 matmuls miscompute on HW), the 180 MB
masking pass on ACT, and the per-candidate dot products fused into single
DVE scalar_tensor_tensor ops:
    s = (ent * mask_col) * alias ; score_col = sum_h(s)
"""

import sys

import numpy as np

if "/opt/trn_rl_repo" not in sys.path:
    sys.path.insert(0, "/opt/trn_rl_repo")

B, N, L = 64, 512, 768
M, K, H = 24, 30, 1024
L2 = 2 * L  # 1536
L2E = 1664  # padded contraction: 1536 span dims + 1 bias row + 127 zeros
NCORES = 8
B_SH = B // NCORES  # 8 batches per core
R = B_SH * M  # 192 alias rows per core
RA, RB = 128, R - 128  # partition split of the 192 rows
ROWS = R * K  # 5760 entity rows per core
KT = L2E // 128  # 13 contraction tiles

_cached_nc = None
TRACE = False
last_results = None


def _build():
    import concourse.bacc as bacc
    import concourse.mybir as mybir
    import concourse.tile as tile

    f32 = mybir.dt.float32
    mult = mybir.AluOpType.mult

    nc = bacc.Bacc(
        "TRN2", target_bir_lowering=False, debug=False, enable_asserts=False
    )

    pair_t = nc.dram_tensor("pair_t", [L2, R], f32, kind="ExternalInput")
    w_span = nc.dram_tensor("w_span", [L2, H], f32, kind="ExternalInput")
    b_row = nc.dram_tensor("b_row", [1, H], f32, kind="ExternalInput")
    mask_keep = nc.dram_tensor("mask_keep", [R, K], f32, kind="ExternalInput")
    ent_in = nc.dram_tensor("ent_in", [ROWS, H], f32, kind="ExternalInput")
    ent_out = nc.dram_tensor("ent_out", [ROWS, H], f32, kind="ExternalOutput")
    scores = nc.dram_tensor("scores", [R, K], f32, kind="ExternalOutput")

    with tile.TileContext(nc) as tc:
        with (
            tc.tile_pool(name="const", bufs=1) as const,
            tc.tile_pool(name="psum", bufs=4, space="PSUM") as psum,
            tc.tile_pool(name="ent_a", bufs=6) as pe_a,
            tc.tile_pool(name="ent_b", bufs=6) as pe_b,
            tc.tile_pool(name="t_a", bufs=6) as pt_a,
            tc.tile_pool(name="t_b", bufs=6) as pt_b,
            tc.tile_pool(name="s_a", bufs=2) as ps_a,
            tc.tile_pool(name="s_b", bufs=2) as ps_b,
        ):
            w_t = const.tile([128, KT * H], f32)
            p_t = const.tile([128, KT * R], f32)
            m_a = const.tile([RA, K], f32)
            m_b = const.tile([RB, K], f32)
            alias_a = const.tile([RA, H], f32)
            alias_b = const.tile([RB, H], f32)
            sc_a = const.tile([RA, K], f32)
            sc_b = const.tile([RB, K], f32)

            wv = w_span[:].rearrange("(kt p) h -> kt p h", p=128)
            pv = pair_t[:].rearrange("(kt p) r -> kt p r", p=128)
            for kt in range(KT - 1):
                nc.sync.dma_start(w_t[:, kt * H : (kt + 1) * H], wv[kt])
                nc.sync.dma_start(p_t[:, kt * R : (kt + 1) * R], pv[kt])
            # 13th contraction tile: row 0 carries the bias (vs a ones row in
            # pair), rows 1-127 are zero. Built on-device to skip 0.6 MB of
            # zero-padding DMA. Rank-1 f32 matmuls miscompute on HW, hence
            # the full-K tile instead of a K=1 bias matmul.
            ob, op = (KT - 1) * H, (KT - 1) * R
            nc.vector.memset(w_t[:, ob : ob + H], 0.0)
            nc.sync.dma_start(w_t[0:1, ob : ob + H], b_row[:])
            nc.vector.memset(p_t[:, op : op + R], 0.0)
            nc.vector.memset(p_t[0:1, op : op + R], 1.0)
            nc.sync.dma_start(m_a[:], mask_keep[0:RA, :])
            nc.sync.dma_start(m_b[:], mask_keep[RA:R, :])

            # alias = pair_ext @ W_ext on PE: out[m, n] = sum_k pair_t[k, m] w[k, n]
            for moff, msz, atile in ((0, RA, alias_a), (RA, RB, alias_b)):
                for ni in range(2):
                    acc = psum.tile([msz, 512], f32)
                    for kt in range(KT):
                        nc.tensor.matmul(
                            acc[:],
                            p_t[:, kt * R + moff : kt * R + moff + msz],
                            w_t[:, kt * H + ni * 512 : kt * H + ni * 512 + 512],
                            start=(kt == 0),
                            stop=(kt == KT - 1),
                        )
                    nc.vector.tensor_copy(atile[:, ni * 512 : ni * 512 + 512], acc[:])

            entv = ent_in[:].rearrange("(r k) h -> k r h", k=K)
            entov = ent_out[:].rearrange("(r k) h -> k r h", k=K)
            for k in range(K):
                ea = pe_a.tile([RA, H], f32)
                eb = pe_b.tile([RB, H], f32)
                nc.sync.dma_start(ea[:], entv[k, 0:RA, :])
                nc.sync.dma_start(eb[:], entv[k, RA:R, :])
                # masked entity rows on ACT (per-partition scale)
                ta = pt_a.tile([RA, H], f32)
                tb = pt_b.tile([RB, H], f32)
                nc.scalar.mul(ta[:], ea[:], m_a[:, k : k + 1])
                nc.scalar.mul(tb[:], eb[:], m_b[:, k : k + 1])
                nc.gpsimd.dma_start(entov[k, 0:RA, :], ta[:])
                nc.gpsimd.dma_start(entov[k, RA:R, :], tb[:])
                # fused mask * dot(ent, alias) on DVE
                sa = ps_a.tile([RA, H], f32)
                sb = ps_b.tile([RB, H], f32)
                nc.vector.scalar_tensor_tensor(
                    out=sa[:],
                    in0=ea[:],
                    scalar=m_a[:, k : k + 1],
                    in1=alias_a[:],
                    op0=mult,
                    op1=mult,
                    accum_out=sc_a[:, k : k + 1],
                )
                nc.vector.scalar_tensor_tensor(
                    out=sb[:],
                    in0=eb[:],
                    scalar=m_b[:, k : k + 1],
                    in1=alias_b[:],
                    op0=mult,
                    op1=mult,
                    accum_out=sc_b[:, k : k + 1],
                )

            nc.sync.dma_start(scores[0:RA, :], sc_a[:])
            nc.sync.dma_start(scores[RA:R, :], sc_b[:])

    nc.compile()
    return nc


def _host_prepare(inputs):
    sent = np.asarray(inputs["sent_embedding"], dtype=np.float32)
    entity = np.asarray(inputs["entity_embedding"], dtype=np.float32)
    mask = np.asarray(inputs["entity_embedding_mask"]).astype(bool)
    sidx = np.asarray(inputs["start_span_idx"]).astype(np.int64)
    eidx = np.asarray(inputs["end_span_idx"]).astype(np.int64)
    w = np.asarray(inputs["W_span"], dtype=np.float32)
    b = np.asarray(inputs["b_span"], dtype=np.float32)

    def gather(idx):
        safe = np.where(idx < 0, 0, idx)
        g = sent[np.arange(B)[:, None], safe]  # (B, M, L)
        return np.where((idx < 0)[:, :, None], np.float32(0.0), g)

    pair = np.concatenate([gather(sidx), gather(eidx)], axis=-1)  # (B, M, 1536)
    maskf = np.where(mask, np.float32(0.0), np.float32(1.0))  # keep-multiplier

    in_maps = []
    for c in range(NCORES):
        sl = slice(c * B_SH, (c + 1) * B_SH)
        in_maps.append(
            {
                "pair_t": np.ascontiguousarray(pair[sl].reshape(R, L2).T),
                "w_span": w,
                "b_row": b.reshape(1, H),
                "mask_keep": maskf[sl].reshape(R, K),
                "ent_in": entity[sl].reshape(ROWS, H),
            }
        )
    return in_maps


def kernel(**inputs):
    global _cached_nc, last_results
    from concourse.bass_utils import run_bass_kernel_spmd

    in_maps = _host_prepare(inputs)
    if _cached_nc is None:
        _cached_nc = _build()

    res = run_bass_kernel_spmd(
        _cached_nc, in_maps, core_ids=list(range(NCORES)), trace=TRACE
    )
    last_results = res
    outs = res.results
    ent = np.stack([outs[c]["ent_out"] for c in range(NCORES)]).reshape(B, M, K, H)
    final_scores = np.stack([outs[c]["scores"] for c in range(NCORES)]).reshape(B, M, K)
    return final_scores, ent
